# revision 12
# baseline (speedup 1.0000x reference)
"""Two-layer GCN (scalar-feature factored form) on 8 Trainium2 NeuronCores.

v2: SBUF-resident message routing via GPSIMD local_scatter + TensorE
transposes, replacing v1's descriptor-per-edge indirect-DMA gathers.

Math (features factor out; x is [N,1], W1 [1,35]):
  deg[v] = indeg+1 (self loop);  dinv = rsqrt(deg)
  p      = dinv * x                               (pass-1 table)
  s1[v]  = dinv[v] * (sum_{u->v} p[u] + p[v])
  q[v]   = dinv[v] * sum_k sigmoid(s1[v]*W1[k]+b1[k]) * W2[k]
  s2[v]  = dinv[v] * (sum_{u->v} q[u] + q[v])
  out[v] = sigmoid(s2[v] + b2)

Per pass, per core (edges with owned dst, plus self edges):
  table [128,784] (rank-major) -> data1/data2 = 2+4 interleaved bf16
  copies (DVE) -> stage A: two local_scatters place each edge copy at
  (src-part r, tile t, col d=dst-part) -> stage B: T TensorE transposes
  flip tiles to (d, t*128+r) -> stage C: two local_scatters route each
  value to its dst node's 32 reserved slots (zero-padded) -> DVE
  tensor_reduce 16-wide virtuals -> pair-reduce -> per-node sums.
  Edges beyond 6 copies/cell go via a 4096-descriptor indirect-DMA
  strip (gather row -> DRAM -> [128,W] reload); nodes with >32
  messages overflow into end-aligned strip virtuals merged post-reduce.
AllGather shares q between the passes. The routing plan tensors are
shared by both passes (identical edge structure and table layout).
"""
import os
import numpy as np
import ml_dtypes

bf16 = ml_dtypes.bfloat16

N_NODES = 100000
N_PAD = 100352            # 128*784 = 8*12544
N_CORES = 8
PER_CORE = 12544          # 128 partitions * 98 nodes
JPP = 98                  # nodes per partition
TCOLS = 784               # table cols per partition
KSLOT = 32                # message slots per node (2 virtuals * 16)
K0 = 16                   # reduce-1 width
WF1 = 28                  # residual strip cols per partition
VF2 = 2                   # overflow strip virtuals per partition
WF2 = VF2 * K0            # overflow strip cols
NFD = 128 * (WF1 + WF2)   # fallback gather descriptors (5632)
FCHUNKS = [(k * 512, 512) for k in range(11)]
CHUNK = 2048
T1CAP = 15
T2CAP = 15

LAST_RESULT = None


def _prep(x, edge_index):
    rng = np.random.default_rng(12345)
    x = np.asarray(x, dtype=np.float32).reshape(-1)
    ei = np.asarray(edge_index)
    src = ei[0].astype(np.int64)
    dst = ei[1].astype(np.int64)

    deg = np.bincount(dst, minlength=N_NODES) + 1
    deg_full = np.ones(N_PAD, np.int64)
    deg_full[:N_NODES] = deg
    x_full = np.zeros(N_PAD, np.float32)
    x_full[:N_NODES] = x
    dinv_full = 1.0 / np.sqrt(deg_full.astype(np.float64))

    order = np.argsort(-deg_full, kind="stable")
    pos = np.empty(N_PAD, np.int64)
    pos[order] = np.arange(N_PAD)
    core_of = pos % N_CORES

    # provisional within-core position (for src-partition estimates)
    iprov = np.empty(N_PAD, np.int64)
    for c in range(N_CORES):
        own = np.where(core_of == c)[0]
        iprov[own] = rng.permutation(len(own))
    P_hat = core_of * 16 + iprov // TCOLS

    # ---- reduce-slot greedy per core (balance n(r,d) cells) ----
    p_of = np.empty(N_PAD, np.int64)   # reduce partition of each node
    j_of = np.empty(N_PAD, np.int64)   # reduce col of each node
    TAU = 14
    for c in range(N_CORES):
        own = order[np.arange(c, N_PAD, N_CORES)]  # degree-desc
        sel = core_of[dst] == c
        es, ed = src[sel], dst[sel]
        o = np.argsort(ed, kind="stable")
        es, ed = es[o], ed[o]
        cnt_in = np.bincount(ed, minlength=N_PAD)
        start_in = np.zeros(N_PAD + 1, np.int64)
        np.cumsum(cnt_in, out=start_in[1:])
        L = np.zeros((128, 128), np.int32)
        nfill = np.zeros(128, np.int64)     # normal nodes placed
        f2cnt = np.zeros(128, np.int64)     # F2 top-slot claims (j=96+s)
        is_f2 = deg_full[own] > KSLOT
        # F2 nodes come first (degree-desc order), claiming j = 96+s;
        # normals fill j = 0..95 then any unclaimed top slots.
        for k in range(len(own)):
            v = own[k]
            rs = P_hat[es[start_in[v]:start_in[v + 1]]]
            Lr = L[rs, :]
            cost = (Lr >= TAU).sum(axis=0) * 100000 + Lr.sum(axis=0)
            if is_f2[k]:
                cost = cost + np.where(f2cnt < VF2, 0, 1 << 30)
                d = int(np.argmin(cost))
                j_of[v] = JPP - VF2 + f2cnt[d]
                f2cnt[d] += 1
            else:
                cost = cost + np.where(nfill < JPP - f2cnt, 0, 1 << 30)
                d = int(np.argmin(cost))
                j = nfill[d]
                if j >= JPP - VF2:
                    # skip over F2-claimed top slots (claims are 96..96+f2cnt)
                    j = j + f2cnt[d]
                assert j < JPP
                j_of[v] = j
                nfill[d] += 1
            p_of[v] = d
            np.add.at(L, (rs, d), 1)

    i_of = p_of * JPP + j_of
    rank = core_of * PER_CORE + i_of
    node_at = np.empty(N_PAD, np.int64)
    node_at[rank] = np.arange(N_PAD)
    P_fin = rank // TCOLS
    X_fin = rank % TCOLS

    xf = x_full[node_at].reshape(128, TCOLS).astype(np.float32)
    degf = deg_full[node_at].reshape(128, TCOLS).astype(np.float32)

    cores = []
    stats = dict(t1=0, t2=0, f1max=0, f1tot=0)
    for c in range(N_CORES):
        sel = core_of[dst] == c
        es = src[sel].copy()
        ed = dst[sel].copy()
        E = len(es)
        r_e = P_fin[es]
        x_e = X_fin[es]
        d_e = p_of[ed]
        rk_e = rank[es]

        # slot index within dst node (message order), F2 = beyond KSLOT
        o = np.lexsort((rng.random(E), ed))
        es, ed, r_e, x_e, d_e, rk_e = (a[o] for a in
                                       (es, ed, r_e, x_e, d_e, rk_e))
        cnt_d = np.bincount(ed, minlength=N_PAD)
        st_d = np.zeros(N_PAD + 1, np.int64)
        np.cumsum(cnt_d, out=st_d[1:])
        kslot = np.arange(E) - st_d[ed]
        f2 = kslot >= KSLOT

        # copy index within src cell (non-F2 edges only)
        nf = ~f2
        idx_nf = np.where(nf)[0]
        es_n = es[idx_nf]
        o2 = np.lexsort((rng.random(len(idx_nf)), es_n))
        idx_nf = idx_nf[o2]
        es_n = es[idx_nf]
        cnt_s = np.bincount(es_n, minlength=N_PAD)
        st_s = np.zeros(N_PAD + 1, np.int64)
        np.cumsum(cnt_s, out=st_s[1:])
        copy_id = np.full(E, -1, np.int64)
        copy_id[idx_nf] = np.arange(len(idx_nf)) - st_s[es_n]

        # route class: 0=A1 (copy 0-1), 1=A2 (copy 2-5), 2=F1, 3=F2
        cls = np.full(E, 2, np.int64)
        cls[f2] = 3
        cls[(copy_id >= 0) & (copy_id < 2)] = 0
        cls[(copy_id >= 2) & (copy_id < 6)] = 1

        # tile assignment per (half, r, d); overflow -> F1
        tile = np.full(E, -1, np.int64)
        for h, cap in ((0, T1CAP), (1, T2CAP)):
            m = np.where(cls == h)[0]
            key = r_e[m] * 128 + d_e[m]
            o3 = np.lexsort((rng.random(len(m)), key))
            m = m[o3]
            key = key[o3]
            stk = np.zeros(128 * 128 + 1, np.int64)
            np.cumsum(np.bincount(key, minlength=128 * 128),
                      out=stk[1:])
            posk = np.arange(len(m)) - stk[key]
            tile[m] = posk
            over = posk >= cap
            cls[m[over]] = 2
            tile[m[over]] = -1
            if h == 0:
                stats["t1"] = max(stats["t1"],
                                  int(posk.max() + 1) if len(posk) else 0)
            else:
                stats["t2"] = max(stats["t2"],
                                  int(posk.max() + 1) if len(posk) else 0)

        # F1 strip cols per dst partition
        m1 = np.where(cls == 2)[0]
        o4 = np.lexsort((rng.random(len(m1)), d_e[m1]))
        m1 = m1[o4]
        stw = np.zeros(129, np.int64)
        np.cumsum(np.bincount(d_e[m1], minlength=128), out=stw[1:])
        w1col = np.arange(len(m1)) - stw[d_e[m1]]
        assert w1col.max(initial=0) < WF1, f"F1 overflow {w1col.max()}"
        stats["f1max"] = max(stats["f1max"], int(w1col.max(initial=-1) + 1))
        stats["f1tot"] += len(m1)

        cores.append(dict(es=es, ed=ed, r=r_e, x=x_e, d=d_e, rk=rk_e,
                          kslot=kslot, cls=cls, copy=copy_id, tile=tile,
                          m1=m1, w1col=w1col))

    T1 = min(T1CAP, max(stats["t1"], 1))
    T2 = min(T2CAP, max(stats["t2"], 1))
    T = T1 + T2
    CSTREAM = T * 128 + WF1

    for c in range(N_CORES):
        cc = cores[c]
        cls, tile = cc["cls"], cc["tile"]
        r_e, x_e, d_e, rk_e = cc["r"], cc["x"], cc["d"], cc["rk"]
        ed, kslot = cc["ed"], cc["kslot"]
        slot_fin = j_of[ed] * KSLOT + kslot  # < 3136 for non-F2

        a1idx = np.full((128, 2 * TCOLS), -1, np.int16)
        a2idx = np.full((128, 4 * TCOLS), -1, np.int16)
        c1idx = np.full((128, CSTREAM), -1, np.int16)
        c2idx = np.full((128, CSTREAM), -1, np.int16)
        fidx_flat = np.full(NFD, N_PAD, np.int64)

        m = cls == 0
        a1idx[r_e[m], 2 * x_e[m] + cc["copy"][m]] = \
            (tile[m] * 128 + d_e[m]).astype(np.int16)
        m = cls == 1
        a2idx[r_e[m], 4 * x_e[m] + (cc["copy"][m] - 2)] = \
            (tile[m] * 128 + d_e[m]).astype(np.int16)
        abs_tile = np.where(cls == 1, tile + T1, tile)

        def place_c(rows, srccol, slots):
            s1m = slots < 1568
            c1idx[rows[s1m], srccol[s1m]] = slots[s1m].astype(np.int16)
            c2idx[rows[~s1m], srccol[~s1m]] = \
                (slots[~s1m] - 1568).astype(np.int16)

        m = (cls == 0) | (cls == 1)
        place_c(d_e[m], abs_tile[m] * 128 + r_e[m], slot_fin[m])
        m1, w1col = cc["m1"], cc["w1col"]
        place_c(d_e[m1], T * 128 + w1col, slot_fin[m1])
        fidx_flat[d_e[m1] * WF1 + w1col] = rk_e[m1]

        # F2: overflow edges -> strip virtuals of dst node's partition
        m2 = np.where(cls == 3)[0]
        vs = j_of[ed[m2]] - (JPP - VF2)   # strip virtual index of node
        assert (vs >= 0).all()
        ks2 = kslot[m2] - KSLOT
        assert ks2.max(initial=0) < K0, "F2 virtual overflow"
        fidx_flat[128 * WF1 + d_e[m2] * WF2 + vs * K0 + ks2] = rk_e[m2]

        fidx = np.empty((128, NFD // 128), np.int32)
        k = np.arange(NFD)
        fidx[k % 128, k // 128] = fidx_flat.astype(np.int32)

        own = order[np.arange(c, N_PAD, N_CORES)]
        dinvo = np.zeros((128, JPP), np.float32)
        dinvo[p_of[own], j_of[own]] = dinv_full[own].astype(np.float32)
        pown = np.zeros((128, JPP), np.float32)
        pown[p_of[own], j_of[own]] = (dinv_full[own] *
                                      x_full[own]).astype(np.float32)
        nodes_i = node_at[c * PER_CORE + np.arange(PER_CORE)]

        cores[c] = dict(a1idx=a1idx, a2idx=a2idx, c1idx=c1idx,
                        c2idx=c2idx, fidx=fidx, dinvo=dinvo, pown=pown,
                        nodes_i=nodes_i)

    return dict(xf=xf, degf=degf, cores=cores, T1=T1, T2=T2, T=T,
                CSTREAM=CSTREAM, stats=stats)


def make_inmaps(meta, W1, b1, W2, b2):
    w1b = np.broadcast_to(np.asarray(W1, np.float32).reshape(1, 35),
                          (128, 35)).copy()
    bb1 = np.broadcast_to(np.asarray(b1, np.float32).reshape(1, 35),
                          (128, 35)).copy()
    w2b = np.broadcast_to(np.asarray(W2, np.float32).reshape(1, 35),
                          (128, 35)).copy()
    bb2 = np.full((128, 1), float(np.asarray(b2).reshape(1)[0]), np.float32)
    ident = np.eye(128, dtype=bf16)
    in_maps = []
    for c in range(N_CORES):
        cc = meta["cores"][c]
        in_maps.append({
            "xf": meta["xf"], "degf": meta["degf"],
            "a1idx": cc["a1idx"], "a2idx": cc["a2idx"],
            "c1idx": cc["c1idx"], "c2idx": cc["c2idx"],
            "fidx": cc["fidx"], "dinvo": cc["dinvo"], "pown": cc["pown"],
            "w1": w1b, "bb1": bb1, "w2": w2b, "bb2": bb2,
            "ident": ident,
        })
    return in_maps


def unshard(meta, per_core_out):
    out_full = np.empty(N_PAD, np.float32)
    for c in range(N_CORES):
        out_full[meta["cores"][c]["nodes_i"]] = \
            np.asarray(per_core_out[c], np.float32).reshape(PER_CORE)
    return out_full[:N_NODES].reshape(N_NODES, 1).astype(np.float32)


def _build_program(meta):
    import concourse.bass as bass
    import concourse.bacc as bacc
    import concourse.mybir as mybir
    from concourse import library_config
    from concourse.bass import IndirectOffsetOnAxis
    from contextlib import ExitStack

    f32 = mybir.dt.float32
    bf = mybir.dt.bfloat16
    i16 = mybir.dt.int16
    i32 = mybir.dt.int32
    AF = mybir.ActivationFunctionType
    OP = mybir.AluOpType

    T1, T2, T, CSTREAM = meta["T1"], meta["T2"], meta["T"], meta["CSTREAM"]
    NV = (3136 + WF2) // K0          # reduce-1 virtuals per partition

    # vs (DVE milestone) schedule
    V_P, V_D0, V_S0, V_SIG1, V_Q, V_D1, V_SS1, V_S2 = range(1, 9)

    nc = bacc.Bacc("TRN2", num_swdge_queues=4)
    xf_d = nc.dram_tensor("xf", [128, TCOLS], f32, kind="ExternalInput")
    degf_d = nc.dram_tensor("degf", [128, TCOLS], f32, kind="ExternalInput")
    a1x_d = nc.dram_tensor("a1idx", [128, 2 * TCOLS], i16, kind="ExternalInput")
    a2x_d = nc.dram_tensor("a2idx", [128, 4 * TCOLS], i16, kind="ExternalInput")
    c1x_d = nc.dram_tensor("c1idx", [128, CSTREAM], i16, kind="ExternalInput")
    c2x_d = nc.dram_tensor("c2idx", [128, CSTREAM], i16, kind="ExternalInput")
    fx_d = nc.dram_tensor("fidx", [128, NFD // 128], i32, kind="ExternalInput")
    dinvo_d = nc.dram_tensor("dinvo", [128, JPP], f32, kind="ExternalInput")
    pown_d = nc.dram_tensor("pown", [128, JPP], f32, kind="ExternalInput")
    w1_d = nc.dram_tensor("w1", [128, 35], f32, kind="ExternalInput")
    bb1_d = nc.dram_tensor("bb1", [128, 35], f32, kind="ExternalInput")
    w2_d = nc.dram_tensor("w2", [128, 35], f32, kind="ExternalInput")
    bb2_d = nc.dram_tensor("bb2", [128, 1], f32, kind="ExternalInput")
    id_d = nc.dram_tensor("ident", [128, 128], bf, kind="ExternalInput")
    outp_d = nc.dram_tensor("outp", [PER_CORE, 1], f32, kind="ExternalOutput")
    DBG = os.environ.get("BASS_DEBUG_DUMP", "0") == "1"
    if DBG:
        dbg_data1 = nc.dram_tensor("dbg_data1", [128, 2 * TCOLS], f32,
                                   kind="ExternalOutput")
        dbg_aout = nc.dram_tensor("dbg_aout", [128, T * 128], f32,
                                  kind="ExternalOutput")
        dbg_cdat = nc.dram_tensor("dbg_cdat", [128, CSTREAM], f32,
                                  kind="ExternalOutput")
        dbg_fin = nc.dram_tensor("dbg_fin", [128, 3136 + WF2], f32,
                                 kind="ExternalOutput")
        dbg_r1 = nc.dram_tensor("dbg_r1", [128, NV], f32,
                                kind="ExternalOutput")
        dbg_s1 = nc.dram_tensor("dbg_s1", [128, JPP], f32,
                                kind="ExternalOutput")
        dbg_x1 = nc.dram_tensor("dbg_x1", [128, JPP], f32,
                                kind="ExternalOutput")
        dbg_x2 = nc.dram_tensor("dbg_x2", [128, JPP], f32,
                                kind="ExternalOutput")
        dbg_x3 = nc.dram_tensor("dbg_x3", [128, JPP], f32,
                                kind="ExternalOutput")

    ptab = nc.dram_tensor("ptab", [N_PAD + 1, 1], f32)
    qown = nc.dram_tensor("qown", [PER_CORE, 1], f32)
    qtab = nc.dram_tensor("qtab", [N_PAD + 1, 1], f32)
    fs1_d = nc.dram_tensor("fstrip1", [NFD, 1], f32)
    fs2_d = nc.dram_tensor("fstrip2", [NFD, 1], f32)

    es = ExitStack()
    _n = [0]
    def sb(shape, dt):
        _n[0] += 1
        return es.enter_context(nc.sbuf_tensor(f"sb{_n[0]}", shape, dt))
    sem = lambda name: es.enter_context(nc.semaphore(name))

    xf_sb = sb([128, TCOLS], f32); degf_sb = sb([128, TCOLS], f32)
    rcp_sb = sb([128, TCOLS], f32); dinvf_sb = sb([128, TCOLS], f32)
    tab_sb = sb([128, TCOLS], f32)       # p (pass1) / q (pass2)
    data1_sb = sb([128, 2 * TCOLS], bf)
    data2_sb = sb([128, 4 * TCOLS], bf)
    a1x_sb = sb([128, 2 * TCOLS], i16); a2x_sb = sb([128, 4 * TCOLS], i16)
    c1x_sb = sb([128, CSTREAM], i16); c2x_sb = sb([128, CSTREAM], i16)
    fx_sb = sb([128, NFD // 128], i32)
    aout_sb = sb([128, T * 128], bf)
    cdat_sb = sb([128, CSTREAM], bf)
    fin_sb = sb([128, 3136 + WF2], bf)
    srow_sb = sb([1, NFD], f32)
    fstr1_sb = sb([128, WF1], f32); fstr2_sb = sb([128, WF2], f32)
    r1_sb = sb([128, NV], f32)
    sraw_sb = sb([128, JPP], f32); s1_sb = sb([128, JPP], f32)
    dinvo_sb = sb([128, JPP], f32); pown_sb = sb([128, JPP], f32)
    sig_sbs = [sb([128, JPP], f32) for _ in range(4)]
    accA_sb = sb([128, JPP], f32); accB_sb = sb([128, JPP], f32)
    qown_sb = sb([128, JPP], f32); out_sb = sb([128, JPP], f32)
    w1_sb = sb([128, 35], f32); bb1_sb = sb([128, 35], f32)
    w2_sb = sb([128, 35], f32); bb2_sb = sb([128, 1], f32)
    id_sb = sb([128, 128], bf)
    zero_sb = sb([1, 1], f32)
    DBGSB = os.environ.get("BASS_DEBUG_DUMP", "0") == "1"
    if DBGSB:
        dbgx1_sb = sb([128, JPP], f32)
        dbgx2_sb = sb([128, JPP], f32)
        dbgx3_sb = sb([128, JPP], f32)
        dbgc1_sb = sb([128, 2 * TCOLS], f32)
        dbgc2_sb = sb([128, T * 128], f32)
        dbgc3_sb = sb([128, CSTREAM], f32)
        dbgc4_sb = sb([128, 3136 + WF2], f32)
    ps = [es.enter_context(nc.psum_tensor(f"ps{k}", [128, 128], bf))
          for k in range(2)]

    ls = sem("ls")      # sync input loads (13 x 16 = 208)
    vs = sem("vs")      # DVE milestones (schedule above)
    va = sem("va")      # DVE reciprocal -> ACT
    ab = sem("ab")      # ACT dinvf -> DVE
    pwa = sem("pwa"); pwv = sem("pwv")   # 35-k pingpong
    as_ = sem("as_")    # ACT final sigmoid
    ga = sem("ga")      # gpsimd A-scatters (1,2 pass1; 3,4 pass2)
    gc = sem("gc")      # gpsimd C-scatters (1,2 pass1; 3,4 pass2)
    ds = sem("ds")      # gpsimd DMA completions
    fs = sem("fs")      # fallback gather/store chain
    f2s = sem("f2s")    # fallback reloads (sync)
    tp = sem("tp")      # tensor transposes (1..2T)
    pv = sem("pv")      # DVE psum drains (1..2T)
    qs = sem("qs")      # qtab reload
    ccs = sem("ccs")

    ptab_v = ptab[0:N_PAD, 0:1].rearrange("(p c) one -> p (c one)", p=128)
    qtab_v = qtab[0:N_PAD, 0:1].rearrange("(p c) one -> p (c one)", p=128)
    qown_v = qown[:, 0:1].rearrange("(p j) one -> p (j one)", p=128)
    outp_v = outp_d[:, 0:1].rearrange("(p j) one -> p (j one)", p=128)
    fs1_row = fs1_d[:, 0:1].rearrange("(o n) one -> o (n one)", o=1)
    fs2_row = fs2_d[:, 0:1].rearrange("(o n) one -> o (n one)", o=1)
    fs1_f1 = fs1_d[0:128 * WF1, 0:1].rearrange("(p w) one -> p (w one)", p=128)
    fs1_f2 = fs1_d[128 * WF1:NFD, 0:1].rearrange("(p w) one -> p (w one)", p=128)
    fs2_f1 = fs2_d[0:128 * WF1, 0:1].rearrange("(p w) one -> p (w one)", p=128)
    fs2_f2 = fs2_d[128 * WF1:NFD, 0:1].rearrange("(p w) one -> p (w one)", p=128)

    NCHUNK = len(FCHUNKS)
    FS_G0 = 16 * NCHUNK            # pass-1 gathers done
    FS_S0 = FS_G0 + 16             # pass-1 strip stored
    FS_G1 = FS_S0 + 16 * NCHUNK
    FS_S1 = FS_G1 + 16

    with es:
      with nc.Block() as block:

        @block.sync
        def _(s):
            for sbuf, dr in ((degf_sb, degf_d), (xf_sb, xf_d),
                             (a1x_sb, a1x_d), (a2x_sb, a2x_d),
                             (id_sb, id_d), (fx_sb, fx_d),
                             (c1x_sb, c1x_d), (c2x_sb, c2x_d),
                             (dinvo_sb, dinvo_d), (pown_sb, pown_d),
                             (w1_sb, w1_d), (bb1_sb, bb1_d),
                             (w2_sb, w2_d), (bb2_sb, bb2_d)):
                s.dma_start(sbuf[:], dr[:]).then_inc(ls, 16)
            s.wait_ge(fs, FS_S0)
            s.dma_start(fstr1_sb[:], fs1_f1).then_inc(f2s, 16)
            s.dma_start(fstr2_sb[:], fs1_f2).then_inc(f2s, 16)
            s.wait_ge(ccs, 1)
            s.dma_start(tab_sb[:], qtab_v).then_inc(qs, 16)
            s.wait_ge(fs, FS_S1)
            s.dma_start(fstr1_sb[:], fs2_f1).then_inc(f2s, 16)
            s.dma_start(fstr2_sb[:], fs2_f2).then_inc(f2s, 16)

        @block.gpsimd
        def _(g):
            g.load_library(library_config.local_scatter)
            g.wait_ge(ls, 96)
            g.memset(zero_sb[:], 0.0)
            g.dma_start(ptab[N_PAD:N_PAD + 1, 0:1], zero_sb[:]).then_inc(ds, 16)
            g.dma_start(qtab[N_PAD:N_PAD + 1, 0:1], zero_sb[:]).then_inc(ds, 16)
            d = 32
            f = 0
            # ---- pass 1 ----
            g.wait_ge(vs, V_P)
            g.dma_start(ptab_v, tab_sb[:]).then_inc(ds, 16); d += 16
            g.wait_ge(ds, d)
            for k, (off, ln) in enumerate(FCHUNKS):
                bi = g.indirect_dma_start(
                    out=srow_sb[0:1, off:off + ln].rearrange(
                        "p (f one) -> p f one", one=1),
                    out_offset=None,
                    in_=ptab[:, :],
                    in_offset=IndirectOffsetOnAxis(
                        ap=fx_sb[:, off // 128:(off + ln) // 128],
                        axis=0),
                )
                bi.ins.queue = f"qPoolDynamic{(k % 4) or ''}"
                bi.then_inc(fs, 16); f += 16
            g.wait_ge(vs, V_D0)
            g.local_scatter(aout_sb[:, 0:T1 * 128], data1_sb[:], a1x_sb[:],
                            channels=128, num_elems=T1 * 128,
                            num_idxs=2 * TCOLS).then_inc(ga, 1)
            g.local_scatter(aout_sb[:, T1 * 128:T * 128], data2_sb[:],
                            a2x_sb[:], channels=128, num_elems=T2 * 128,
                            num_idxs=4 * TCOLS).then_inc(ga, 1)
            g.wait_ge(fs, f)
            g.dma_start(fs1_row, srow_sb[:]).then_inc(fs, 16); f += 16
            g.wait_ge(ls, 128)
            g.wait_ge(pv, T)
            g.wait_ge(vs, V_S0)
            g.local_scatter(fin_sb[:, 0:1568], cdat_sb[:], c1x_sb[:],
                            channels=128, num_elems=1568,
                            num_idxs=CSTREAM).then_inc(gc, 1)
            g.local_scatter(fin_sb[:, 1568:3136], cdat_sb[:], c2x_sb[:],
                            channels=128, num_elems=1568,
                            num_idxs=CSTREAM).then_inc(gc, 1)
            g.wait_ge(vs, V_Q)
            if DBG:
                for dr, sbuf in ((dbg_data1, dbgc1_sb), (dbg_aout, dbgc2_sb),
                                 (dbg_cdat, dbgc3_sb), (dbg_fin, dbgc4_sb),
                                 (dbg_r1, r1_sb), (dbg_s1, s1_sb),
                                 (dbg_x1, dbgx1_sb), (dbg_x2, dbgx2_sb),
                                 (dbg_x3, dbgx3_sb)):
                    g.dma_start(dr[:], sbuf[:]).then_inc(ds, 16); d += 16
            g.dma_start(qown_v, qown_sb[:]).then_inc(ds, 16); d += 16
            g.wait_ge(ds, d)
            g.collective_compute(
                "AllGather", OP.bypass,
                replica_groups=[list(range(N_CORES))],
                ins=[qown[:, 0:1]],
                outs=[qtab[0:N_PAD, 0:1]],
            ).then_inc(ccs, 1)
            # ---- pass 2 ----
            g.wait_ge(ccs, 1)
            g.wait_ge(fs, FS_S0)
            for k, (off, ln) in enumerate(FCHUNKS):
                bi = g.indirect_dma_start(
                    out=srow_sb[0:1, off:off + ln].rearrange(
                        "p (f one) -> p f one", one=1),
                    out_offset=None,
                    in_=qtab[:, :],
                    in_offset=IndirectOffsetOnAxis(
                        ap=fx_sb[:, off // 128:(off + ln) // 128],
                        axis=0),
                )
                bi.ins.queue = f"qPoolDynamic{(k % 4) or ''}"
                bi.then_inc(fs, 16); f += 16
            g.wait_ge(vs, V_D1)
            g.wait_ge(tp, T)
            g.local_scatter(aout_sb[:, 0:T1 * 128], data1_sb[:], a1x_sb[:],
                            channels=128, num_elems=T1 * 128,
                            num_idxs=2 * TCOLS).then_inc(ga, 1)
            g.local_scatter(aout_sb[:, T1 * 128:T * 128], data2_sb[:],
                            a2x_sb[:], channels=128, num_elems=T2 * 128,
                            num_idxs=4 * TCOLS).then_inc(ga, 1)
            g.wait_ge(fs, f)
            g.dma_start(fs2_row, srow_sb[:]).then_inc(fs, 16); f += 16
            g.wait_ge(pv, 2 * T)
            g.wait_ge(vs, V_SS1)
            g.local_scatter(fin_sb[:, 0:1568], cdat_sb[:], c1x_sb[:],
                            channels=128, num_elems=1568,
                            num_idxs=CSTREAM).then_inc(gc, 1)
            g.local_scatter(fin_sb[:, 1568:3136], cdat_sb[:], c2x_sb[:],
                            channels=128, num_elems=1568,
                            num_idxs=CSTREAM).then_inc(gc, 1)
            g.wait_ge(as_, 1)
            g.dma_start(outp_v, out_sb[:]).then_inc(ds, 16); d += 16
            g.wait_ge(ds, d)

        @block.tensor
        def _(t):
            t.wait_ge(ls, 80)
            for ph in range(2):
                for ti in range(T):
                    t.wait_ge(ga, 2 * ph + (1 if ti < T1 else 2))
                    k = ph * T + ti
                    if k >= 2:
                        t.wait_ge(pv, k - 1)
                    t.matmul(ps[k % 2][:],
                             aout_sb[:, ti * 128:(ti + 1) * 128],
                             id_sb[:], is_transpose=True).then_inc(tp, 1)

        @block.vector
        def _(v):
            v.wait_ge(ls, 32)
            v.reciprocal(rcp_sb[:], degf_sb[:]).then_inc(va, 1)
            v.wait_ge(ab, 1)
            v.tensor_tensor(out=tab_sb[:], in0=dinvf_sb[:], in1=xf_sb[:],
                            op=OP.mult).then_inc(vs, 1)          # V_P
            for ph in range(2):
                if ph == 1:
                    v.wait_ge(qs, 16)
                d1v = data1_sb[:].rearrange("p (x two) -> p x two", two=2)
                d2v = data2_sb[:].rearrange("p (x four) -> p x four", four=4)
                for j in range(2):
                    v.tensor_scalar_add(d1v[:, :, j:j + 1], tab_sb[:], 0.0)
                for j in range(4):
                    ins = v.tensor_scalar_add(d2v[:, :, j:j + 1],
                                              tab_sb[:], 0.0)
                ins.then_inc(vs, 1)                # V_D0 / V_D1
                for ti in range(T):
                    k = ph * T + ti
                    v.wait_ge(tp, k + 1)
                    v.tensor_scalar_add(cdat_sb[:, ti * 128:(ti + 1) * 128],
                                        ps[k % 2][:], 0.0).then_inc(pv, 1)
                v.wait_ge(f2s, 32 * (ph + 1))
                v.tensor_scalar_add(cdat_sb[:, T * 128:T * 128 + WF1],
                                    fstr1_sb[:], 0.0)
                v.tensor_scalar_add(fin_sb[:, 3136:3136 + WF2],
                                    fstr2_sb[:], 0.0).then_inc(vs, 1)
                # V_S0 / V_SS1
                v.wait_ge(ls, 224)
                v.wait_ge(gc, 2 * (ph + 1))
                v.tensor_reduce(
                    out=r1_sb[:],
                    in_=fin_sb[:].rearrange("p (nv k) -> p nv k", k=K0),
                    axis=mybir.AxisListType.X, op=OP.add)
                v.tensor_reduce(
                    out=sraw_sb[:],
                    in_=r1_sb[:, 0:2 * JPP].rearrange(
                        "p (j two) -> p j two", two=2),
                    axis=mybir.AxisListType.X, op=OP.add)
                if DBGSB and ph == 0:
                    v.tensor_scalar_add(dbgx1_sb[:], sraw_sb[:], 0.0)
                v.drain()
                v.tensor_tensor(out=sraw_sb[:, JPP - VF2:JPP],
                                in0=sraw_sb[:, JPP - VF2:JPP],
                                in1=r1_sb[:, 2 * JPP:2 * JPP + VF2],
                                op=OP.add)
                if DBGSB and ph == 0:
                    v.tensor_scalar_add(dbgx2_sb[:], sraw_sb[:], 0.0)
                v.tensor_tensor(out=sraw_sb[:], in0=sraw_sb[:],
                                in1=(pown_sb if ph == 0 else qown_sb)[:],
                                op=OP.add)
                if DBGSB and ph == 0:
                    v.tensor_scalar_add(dbgx3_sb[:], sraw_sb[:], 0.0)
                v.tensor_tensor(out=s1_sb[:], in0=sraw_sb[:],
                                in1=dinvo_sb[:],
                                op=OP.mult).then_inc(vs, 1)  # V_SIG1 / V_S2
                if ph == 0:
                    for k in range(35):
                        sig = sig_sbs[k % 4]
                        v.wait_ge(pwa, k + 1)
                        if k == 0:
                            v.tensor_scalar_mul(accA_sb[:], sig[:],
                                                w2_sb[:, 0:1]).then_inc(pwv, 1)
                        else:
                            sa = accA_sb if k % 2 == 1 else accB_sb
                            da = accB_sb if k % 2 == 1 else accA_sb
                            v.scalar_tensor_tensor(
                                out=da[:], in0=sig[:],
                                scalar=w2_sb[:, k:k + 1],
                                in1=sa[:], op0=OP.mult,
                                op1=OP.add).then_inc(pwv, 1)
                    if DBGSB:
                        v.tensor_scalar_add(dbgc1_sb[:], data1_sb[:], 0.0)
                        v.tensor_scalar_add(dbgc2_sb[:], aout_sb[:], 0.0)
                        v.tensor_scalar_add(dbgc3_sb[:], cdat_sb[:], 0.0)
                        v.tensor_scalar_add(dbgc4_sb[:], fin_sb[:], 0.0)
                    v.tensor_tensor(out=qown_sb[:], in0=accA_sb[:],
                                    in1=dinvo_sb[:],
                                    op=OP.mult).then_inc(vs, 1)   # V_Q

        @block.scalar
        def _(a):
            a.wait_ge(va, 1)
            a.activation(dinvf_sb[:], rcp_sb[:], AF.Sqrt).then_inc(ab, 1)
            a.wait_ge(vs, V_SIG1)
            a.wait_ge(ls, 224)
            for k in range(35):
                buf = sig_sbs[k % 4]
                if k >= 4:
                    a.wait_ge(pwv, k - 3)
                a.activation(buf[:], s1_sb[:], AF.Sigmoid,
                             bias=bb1_sb[:, k:k + 1],
                             scale=w1_sb[:, k:k + 1]).then_inc(pwa, 1)
            a.wait_ge(vs, V_S2)
            a.activation(out_sb[:], s1_sb[:], AF.Sigmoid,
                         bias=bb2_sb[:, 0:1]).then_inc(as_, 1)

    nc.compile()
    return nc


def kernel(x, edge_index, W1, b1, W2, b2):
    global LAST_RESULT
    from concourse.bass_utils import run_bass_kernel_spmd

    meta = _prep(x, edge_index)
    nc = _build_program(meta)
    in_maps = make_inmaps(meta, W1, b1, W2, b2)

    trace = os.environ.get("BASS_KERNEL_TRACE", "0") == "1"
    res = run_bass_kernel_spmd(nc, in_maps, list(range(N_CORES)), trace=trace)
    LAST_RESULT = res
    return unshard(meta, [res.results[c]["outp"] for c in range(N_CORES)])


# revision 16
# speedup vs baseline: 1.1409x; 1.1409x over previous
"""Two-layer GCN (scalar-feature factored form) on 8 Trainium2 NeuronCores.

v2: SBUF-resident message routing via GPSIMD local_scatter + TensorE
transposes, replacing v1's descriptor-per-edge indirect-DMA gathers.

Math (features factor out; x is [N,1], W1 [1,35]):
  deg[v] = indeg+1 (self loop);  dinv = rsqrt(deg)
  p      = dinv * x                               (pass-1 table)
  s1[v]  = dinv[v] * (sum_{u->v} p[u] + p[v])
  q[v]   = dinv[v] * sum_k sigmoid(s1[v]*W1[k]+b1[k]) * W2[k]
  s2[v]  = dinv[v] * (sum_{u->v} q[u] + q[v])
  out[v] = sigmoid(s2[v] + b2)

Per pass, per core (edges with owned dst, plus self edges):
  table [128,784] (rank-major) -> data1/data2 = 2+4 interleaved bf16
  copies (DVE) -> stage A: two local_scatters place each edge copy at
  (src-part r, tile t, col d=dst-part) -> stage B: T TensorE transposes
  flip tiles to (d, t*128+r) -> stage C: two local_scatters route each
  value to its dst node's 32 reserved slots (zero-padded) -> DVE
  tensor_reduce 16-wide virtuals -> pair-reduce -> per-node sums.
  Edges beyond 6 copies/cell go via a 4096-descriptor indirect-DMA
  strip (gather row -> DRAM -> [128,W] reload); nodes with >32
  messages overflow into end-aligned strip virtuals merged post-reduce.
AllGather shares q between the passes. The routing plan tensors are
shared by both passes (identical edge structure and table layout).
"""
import os
import numpy as np
import ml_dtypes

bf16 = ml_dtypes.bfloat16

N_NODES = 100000
N_PAD = 100352            # 128*784 = 8*12544
N_CORES = 8
PER_CORE = 12544          # 128 partitions * 98 nodes
JPP = 98                  # nodes per partition
TCOLS = 784               # table cols per partition
KSLOT = 32                # message slots per node (2 virtuals * 16)
K0 = 16                   # reduce-1 width
WF1 = 28                  # residual strip cols per partition
VF2 = 1                   # overflow strip virtuals per partition
WF2 = VF2 * K0            # overflow strip cols
NFD = 128 * (WF1 + WF2)   # fallback gather descriptors (5632)
FCHUNKS = [(0, 2048), (2048, 2048), (4096, 1536)]
CHUNK = 2048
T1CAP = 15
T2CAP = 15

LAST_RESULT = None


def _prep(x, edge_index):
    rng = np.random.default_rng(12345)
    x = np.asarray(x, dtype=np.float32).reshape(-1)
    ei = np.asarray(edge_index)
    src = ei[0].astype(np.int64)
    dst = ei[1].astype(np.int64)

    deg = np.bincount(dst, minlength=N_NODES) + 1
    deg_full = np.ones(N_PAD, np.int64)
    deg_full[:N_NODES] = deg
    x_full = np.zeros(N_PAD, np.float32)
    x_full[:N_NODES] = x
    dinv_full = 1.0 / np.sqrt(deg_full.astype(np.float64))

    order = np.argsort(-deg_full, kind="stable")
    pos = np.empty(N_PAD, np.int64)
    pos[order] = np.arange(N_PAD)
    core_of = pos % N_CORES

    # provisional within-core position (for src-partition estimates)
    iprov = np.empty(N_PAD, np.int64)
    for c in range(N_CORES):
        own = np.where(core_of == c)[0]
        iprov[own] = rng.permutation(len(own))
    P_hat = core_of * 16 + iprov // TCOLS

    # ---- reduce-slot greedy per core (balance n(r,d) cells) ----
    p_of = np.empty(N_PAD, np.int64)   # reduce partition of each node
    j_of = np.empty(N_PAD, np.int64)   # reduce col of each node
    TAU = 14
    for c in range(N_CORES):
        own = order[np.arange(c, N_PAD, N_CORES)]  # degree-desc
        sel = core_of[dst] == c
        es, ed = src[sel], dst[sel]
        o = np.argsort(ed, kind="stable")
        es, ed = es[o], ed[o]
        cnt_in = np.bincount(ed, minlength=N_PAD)
        start_in = np.zeros(N_PAD + 1, np.int64)
        np.cumsum(cnt_in, out=start_in[1:])
        L = np.zeros((128, 128), np.int32)
        nfill = np.zeros(128, np.int64)     # normal nodes placed
        f2cnt = np.zeros(128, np.int64)     # F2 top-slot claims (j=96+s)
        is_f2 = deg_full[own] > KSLOT
        # F2 nodes come first (degree-desc order), claiming j = 96+s;
        # normals fill j = 0..95 then any unclaimed top slots.
        for k in range(len(own)):
            v = own[k]
            rs = P_hat[es[start_in[v]:start_in[v + 1]]]
            Lr = L[rs, :]
            cost = (Lr >= TAU).sum(axis=0) * 100000 + Lr.sum(axis=0)
            if is_f2[k]:
                cost = cost + np.where(f2cnt < VF2, 0, 1 << 30)
                d = int(np.argmin(cost))
                j_of[v] = JPP - VF2 + f2cnt[d]
                f2cnt[d] += 1
            else:
                cost = cost + np.where(nfill < JPP - f2cnt, 0, 1 << 30)
                d = int(np.argmin(cost))
                j = nfill[d]
                if j >= JPP - VF2:
                    # skip over F2-claimed top slots (claims are 96..96+f2cnt)
                    j = j + f2cnt[d]
                assert j < JPP
                j_of[v] = j
                nfill[d] += 1
            p_of[v] = d
            np.add.at(L, (rs, d), 1)

    i_of = p_of * JPP + j_of
    rank = core_of * PER_CORE + i_of
    node_at = np.empty(N_PAD, np.int64)
    node_at[rank] = np.arange(N_PAD)
    P_fin = rank // TCOLS
    X_fin = rank % TCOLS

    p_rank = (dinv_full * x_full)[node_at].astype(np.float32)  # rank-major
    p_tab = p_rank.reshape(128, TCOLS)
    data1 = np.empty((128, 2 * TCOLS), bf16)
    data2 = np.empty((128, 4 * TCOLS), bf16)
    for j in range(2):
        data1[:, j::2] = p_tab.astype(bf16)
    for j in range(4):
        data2[:, j::4] = p_tab.astype(bf16)

    cores = []
    stats = dict(t1=0, t2=0, f1max=0, f1tot=0)
    for c in range(N_CORES):
        sel = core_of[dst] == c
        es = src[sel].copy()
        ed = dst[sel].copy()
        E = len(es)
        r_e = P_fin[es]
        x_e = X_fin[es]
        d_e = p_of[ed]
        rk_e = rank[es]

        # slot index within dst node (message order), F2 = beyond KSLOT
        o = np.lexsort((rng.random(E), ed))
        es, ed, r_e, x_e, d_e, rk_e = (a[o] for a in
                                       (es, ed, r_e, x_e, d_e, rk_e))
        cnt_d = np.bincount(ed, minlength=N_PAD)
        st_d = np.zeros(N_PAD + 1, np.int64)
        np.cumsum(cnt_d, out=st_d[1:])
        kslot = np.arange(E) - st_d[ed]
        f2 = kslot >= KSLOT

        # copy index within src cell (non-F2 edges only)
        nf = ~f2
        idx_nf = np.where(nf)[0]
        es_n = es[idx_nf]
        o2 = np.lexsort((rng.random(len(idx_nf)), es_n))
        idx_nf = idx_nf[o2]
        es_n = es[idx_nf]
        cnt_s = np.bincount(es_n, minlength=N_PAD)
        st_s = np.zeros(N_PAD + 1, np.int64)
        np.cumsum(cnt_s, out=st_s[1:])
        copy_id = np.full(E, -1, np.int64)
        copy_id[idx_nf] = np.arange(len(idx_nf)) - st_s[es_n]

        # route class: 0=A1 (copy 0-1), 1=A2 (copy 2-5), 2=F1, 3=F2
        cls = np.full(E, 2, np.int64)
        cls[f2] = 3
        cls[(copy_id >= 0) & (copy_id < 2)] = 0
        cls[(copy_id >= 2) & (copy_id < 6)] = 1

        # tile assignment per (half, r, d); overflow -> F1
        tile = np.full(E, -1, np.int64)
        for h, cap in ((0, T1CAP), (1, T2CAP)):
            m = np.where(cls == h)[0]
            key = r_e[m] * 128 + d_e[m]
            o3 = np.lexsort((rng.random(len(m)), key))
            m = m[o3]
            key = key[o3]
            stk = np.zeros(128 * 128 + 1, np.int64)
            np.cumsum(np.bincount(key, minlength=128 * 128),
                      out=stk[1:])
            posk = np.arange(len(m)) - stk[key]
            tile[m] = posk
            over = posk >= cap
            cls[m[over]] = 2
            tile[m[over]] = -1
            if h == 0:
                stats["t1"] = max(stats["t1"],
                                  int(posk.max() + 1) if len(posk) else 0)
            else:
                stats["t2"] = max(stats["t2"],
                                  int(posk.max() + 1) if len(posk) else 0)

        # F1 strip cols per dst partition
        m1 = np.where(cls == 2)[0]
        o4 = np.lexsort((rk_e[m1], d_e[m1]))   # rank-sorted per partition
        m1 = m1[o4]
        stw = np.zeros(129, np.int64)
        np.cumsum(np.bincount(d_e[m1], minlength=128), out=stw[1:])
        w1col = np.arange(len(m1)) - stw[d_e[m1]]
        assert w1col.max(initial=0) < WF1, f"F1 overflow {w1col.max()}"
        stats["f1max"] = max(stats["f1max"], int(w1col.max(initial=-1) + 1))
        stats["f1tot"] += len(m1)

        cores.append(dict(es=es, ed=ed, r=r_e, x=x_e, d=d_e, rk=rk_e,
                          kslot=kslot, cls=cls, copy=copy_id, tile=tile,
                          m1=m1, w1col=w1col))

    T1 = min(T1CAP, max(stats["t1"], 1))
    T2 = min(T2CAP, max(stats["t2"], 1))
    T = T1 + T2
    CSTREAM = T * 128 + WF1

    for c in range(N_CORES):
        cc = cores[c]
        cls, tile = cc["cls"], cc["tile"]
        r_e, x_e, d_e, rk_e = cc["r"], cc["x"], cc["d"], cc["rk"]
        ed, kslot = cc["ed"], cc["kslot"]
        slot_fin = j_of[ed] * KSLOT + kslot  # < 3136 for non-F2

        a1idx = np.full((128, 2 * TCOLS), -1, np.int16)
        a2idx = np.full((128, 4 * TCOLS), -1, np.int16)
        c1idx = np.full((128, CSTREAM), -1, np.int16)
        c2idx = np.full((128, CSTREAM), -1, np.int16)
        fidx_flat = np.full(NFD, N_PAD, np.int64)

        m = cls == 0
        a1idx[r_e[m], 2 * x_e[m] + cc["copy"][m]] = \
            (tile[m] * 128 + d_e[m]).astype(np.int16)
        m = cls == 1
        a2idx[r_e[m], 4 * x_e[m] + (cc["copy"][m] - 2)] = \
            (tile[m] * 128 + d_e[m]).astype(np.int16)
        abs_tile = np.where(cls == 1, tile + T1, tile)

        def place_c(rows, srccol, slots):
            s1m = slots < 1568
            c1idx[rows[s1m], srccol[s1m]] = slots[s1m].astype(np.int16)
            c2idx[rows[~s1m], srccol[~s1m]] = \
                (slots[~s1m] - 1568).astype(np.int16)

        m = (cls == 0) | (cls == 1)
        place_c(d_e[m], abs_tile[m] * 128 + r_e[m], slot_fin[m])
        m1, w1col = cc["m1"], cc["w1col"]
        place_c(d_e[m1], T * 128 + w1col, slot_fin[m1])
        fidx_flat[d_e[m1] * WF1 + w1col] = rk_e[m1]

        # F2: overflow edges -> strip virtuals of dst node's partition
        m2 = np.where(cls == 3)[0]
        vs = j_of[ed[m2]] - (JPP - VF2)   # strip virtual index of node
        assert (vs >= 0).all()
        ks2 = kslot[m2] - KSLOT
        assert ks2.max(initial=0) < K0, "F2 virtual overflow"
        fidx_flat[128 * WF1 + d_e[m2] * WF2 + vs * K0 + ks2] = rk_e[m2]

        fidx = np.empty((128, NFD // 128), np.int32)
        k = np.arange(NFD)
        fidx[k % 128, k // 128] = fidx_flat.astype(np.int32)

        own = order[np.arange(c, N_PAD, N_CORES)]
        dinvo = np.zeros((128, JPP), np.float32)
        dinvo[p_of[own], j_of[own]] = dinv_full[own].astype(np.float32)
        pown = np.zeros((128, JPP), np.float32)
        pown[p_of[own], j_of[own]] = (dinv_full[own] *
                                      x_full[own]).astype(np.float32)
        nodes_i = node_at[c * PER_CORE + np.arange(PER_CORE)]

        pz = np.append(p_rank, 0.0)
        strip = pz[np.minimum(fidx_flat, N_PAD)]
        strip_c = strip[0:128 * WF1].reshape(128, WF1).astype(bf16)
        strip_f = strip[128 * WF1:].reshape(128, WF2).astype(bf16)
        cores[c] = dict(a1idx=a1idx, a2idx=a2idx, c1idx=c1idx,
                        c2idx=c2idx, fidx=fidx, dinvo=dinvo, pown=pown,
                        strip_c=strip_c, strip_f=strip_f,
                        nodes_i=nodes_i)

    return dict(data1=data1, data2=data2, cores=cores, T1=T1, T2=T2, T=T,
                CSTREAM=CSTREAM, stats=stats)


def make_inmaps(meta, W1, b1, W2, b2):
    w1b = np.broadcast_to(np.asarray(W1, np.float32).reshape(1, 35),
                          (128, 35)).copy()
    bb1 = np.broadcast_to(np.asarray(b1, np.float32).reshape(1, 35),
                          (128, 35)).copy()
    w2b = np.broadcast_to(np.asarray(W2, np.float32).reshape(1, 35),
                          (128, 35)).copy()
    bb2 = np.full((128, 1), float(np.asarray(b2).reshape(1)[0]), np.float32)
    ident = np.eye(128, dtype=bf16)
    in_maps = []
    for c in range(N_CORES):
        cc = meta["cores"][c]
        in_maps.append({
            "data1": meta["data1"], "data2": meta["data2"],
            "a1idx": cc["a1idx"], "a2idx": cc["a2idx"],
            "c1idx": cc["c1idx"], "c2idx": cc["c2idx"],
            "fidx": cc["fidx"], "dinvo": cc["dinvo"], "pown": cc["pown"],
            "strip_c": cc["strip_c"], "strip_f": cc["strip_f"],
            "w1": w1b, "bb1": bb1, "w2": w2b, "bb2": bb2,
            "ident": ident,
        })
    return in_maps


def unshard(meta, per_core_out):
    out_full = np.empty(N_PAD, np.float32)
    for c in range(N_CORES):
        out_full[meta["cores"][c]["nodes_i"]] = \
            np.asarray(per_core_out[c], np.float32).reshape(PER_CORE)
    return out_full[:N_NODES].reshape(N_NODES, 1).astype(np.float32)


def _build_program(meta):
    import concourse.bass as bass
    import concourse.bacc as bacc
    import concourse.mybir as mybir
    from concourse import library_config
    from concourse.bass import IndirectOffsetOnAxis
    from contextlib import ExitStack

    f32 = mybir.dt.float32
    bf = mybir.dt.bfloat16
    i16 = mybir.dt.int16
    i32 = mybir.dt.int32
    AF = mybir.ActivationFunctionType
    OP = mybir.AluOpType

    T1, T2, T, CSTREAM = meta["T1"], meta["T2"], meta["T"], meta["CSTREAM"]
    NV = (3136 + WF2) // K0

    # vs (DVE milestone) schedule
    V_SIG1, V_Q, V_D1, V_SS1, V_S2 = range(1, 6)

    nc = bacc.Bacc("TRN2", num_swdge_queues=4)
    d1_d = nc.dram_tensor("data1", [128, 2 * TCOLS], bf, kind="ExternalInput")
    d2_d = nc.dram_tensor("data2", [128, 4 * TCOLS], bf, kind="ExternalInput")
    a1x_d = nc.dram_tensor("a1idx", [128, 2 * TCOLS], i16, kind="ExternalInput")
    a2x_d = nc.dram_tensor("a2idx", [128, 4 * TCOLS], i16, kind="ExternalInput")
    id_d = nc.dram_tensor("ident", [128, 128], bf, kind="ExternalInput")
    fx_d = nc.dram_tensor("fidx", [128, NFD // 128], i32, kind="ExternalInput")
    c1x_d = nc.dram_tensor("c1idx", [128, CSTREAM], i16, kind="ExternalInput")
    c2x_d = nc.dram_tensor("c2idx", [128, CSTREAM], i16, kind="ExternalInput")
    sc_d = nc.dram_tensor("strip_c", [128, WF1], bf, kind="ExternalInput")
    sf_d = nc.dram_tensor("strip_f", [128, WF2], bf, kind="ExternalInput")
    dinvo_d = nc.dram_tensor("dinvo", [128, JPP], f32, kind="ExternalInput")
    pown_d = nc.dram_tensor("pown", [128, JPP], f32, kind="ExternalInput")
    w1_d = nc.dram_tensor("w1", [128, 35], f32, kind="ExternalInput")
    bb1_d = nc.dram_tensor("bb1", [128, 35], f32, kind="ExternalInput")
    w2_d = nc.dram_tensor("w2", [128, 35], f32, kind="ExternalInput")
    bb2_d = nc.dram_tensor("bb2", [128, 1], f32, kind="ExternalInput")
    outp_d = nc.dram_tensor("outp", [PER_CORE, 1], f32, kind="ExternalOutput")

    qown = nc.dram_tensor("qown", [PER_CORE, 1], f32)
    qtab = nc.dram_tensor("qtab", [N_PAD + 1, 1], f32)
    fs2_d = nc.dram_tensor("fstrip2", [NFD, 1], f32)

    es = ExitStack()
    _n = [0]
    def sb(shape, dt):
        _n[0] += 1
        return es.enter_context(nc.sbuf_tensor(f"sb{_n[0]}", shape, dt))
    sem = lambda name: es.enter_context(nc.semaphore(name))

    tab_sb = sb([128, TCOLS], f32)       # q table (pass 2)
    data1_sb = sb([128, 2 * TCOLS], bf)
    data2_sb = sb([128, 4 * TCOLS], bf)
    a1x_sb = sb([128, 2 * TCOLS], i16); a2x_sb = sb([128, 4 * TCOLS], i16)
    c1x_sb = sb([128, CSTREAM], i16); c2x_sb = sb([128, CSTREAM], i16)
    fx_sb = sb([128, NFD // 128], i32)
    aout_sb = sb([128, T * 128], bf)
    cdat_sb = sb([128, CSTREAM], bf)
    fin_sb = sb([128, 3136 + WF2], bf)
    srow_sb = sb([1, NFD], f32)
    fstr1_sb = sb([128, WF1], f32); fstr2_sb = sb([128, WF2], f32)
    r1_sb = sb([128, NV], f32)
    sraw_sb = sb([128, JPP], f32); s1_sb = sb([128, JPP], f32)
    dinvo_sb = sb([128, JPP], f32); pown_sb = sb([128, JPP], f32)
    sig_sbs = [sb([128, JPP], f32) for _ in range(4)]
    accA_sb = sb([128, JPP], f32); accB_sb = sb([128, JPP], f32)
    qown_sb = sb([128, JPP], f32); out_sb = sb([128, JPP], f32)
    w1_sb = sb([128, 35], f32); bb1_sb = sb([128, 35], f32)
    w2_sb = sb([128, 35], f32); bb2_sb = sb([128, 1], f32)
    id_sb = sb([128, 128], bf)
    zero_sb = sb([1, 1], f32)
    ps = [es.enter_context(nc.psum_tensor(f"ps{k}", [128, 128], bf))
          for k in range(2)]

    ls = sem("ls")      # sync input loads (16 x 16 = 256)
    vs = sem("vs")      # DVE milestones
    pwa = sem("pwa"); pwv = sem("pwv")
    as_ = sem("as_")
    ga = sem("ga"); gc = sem("gc")
    ds = sem("ds")
    fs = sem("fs"); f2s = sem("f2s")
    tp = sem("tp"); pv = sem("pv")
    qs = sem("qs"); ccs = sem("ccs")

    qtab_v = qtab[0:N_PAD, 0:1].rearrange("(p c) one -> p (c one)", p=128)
    qown_v = qown[:, 0:1].rearrange("(p j) one -> p (j one)", p=128)
    outp_v = outp_d[:, 0:1].rearrange("(p j) one -> p (j one)", p=128)
    fs2_row = fs2_d[:, 0:1].rearrange("(o n) one -> o (n one)", o=1)
    fs2_f1 = fs2_d[0:128 * WF1, 0:1].rearrange("(p w) one -> p (w one)", p=128)
    fs2_f2 = fs2_d[128 * WF1:NFD, 0:1].rearrange("(p w) one -> p (w one)", p=128)

    NCHUNK = len(FCHUNKS)
    FS_G1 = 16 * NCHUNK
    FS_S1 = FS_G1 + 16

    with es:
      with nc.Block() as block:

        @block.sync
        def _(s):
            loads = [(data1_sb[:], d1_d[:]), (data2_sb[:], d2_d[:]),
                     (a1x_sb[:], a1x_d[:]), (a2x_sb[:], a2x_d[:]),
                     (id_sb[:], id_d[:]), (fx_sb[:], fx_d[:]),
                     (c1x_sb[:], c1x_d[:]), (c2x_sb[:], c2x_d[:]),
                     (cdat_sb[:, T * 128:T * 128 + WF1], sc_d[:]),
                     (fin_sb[:, 3136:3136 + WF2], sf_d[:]),
                     (dinvo_sb[:], dinvo_d[:]), (pown_sb[:], pown_d[:]),
                     (w1_sb[:], w1_d[:]), (bb1_sb[:], bb1_d[:]),
                     (w2_sb[:], w2_d[:]), (bb2_sb[:], bb2_d[:])]
            for dst_ap, src_ap in loads:
                s.dma_start(dst_ap, src_ap).then_inc(ls, 16)
            s.wait_ge(ccs, 1)
            s.dma_start(tab_sb[:], qtab_v).then_inc(qs, 16)
            s.wait_ge(fs, FS_S1)
            s.dma_start(fstr1_sb[:], fs2_f1).then_inc(f2s, 16)
            s.dma_start(fstr2_sb[:], fs2_f2).then_inc(f2s, 16)

        @block.gpsimd
        def _(g):
            g.load_library(library_config.local_scatter)
            g.wait_ge(ls, 128)
            g.memset(zero_sb[:], 0.0)
            g.dma_start(qtab[N_PAD:N_PAD + 1, 0:1], zero_sb[:]).then_inc(ds, 16)
            d = 16
            f = 0
            # ---- pass 1 (all inputs CPU-staged) ----
            g.local_scatter(aout_sb[:, 0:T1 * 128], data1_sb[:], a1x_sb[:],
                            channels=128, num_elems=T1 * 128,
                            num_idxs=2 * TCOLS).then_inc(ga, 1)
            g.local_scatter(aout_sb[:, T1 * 128:T * 128], data2_sb[:],
                            a2x_sb[:], channels=128, num_elems=T2 * 128,
                            num_idxs=4 * TCOLS).then_inc(ga, 1)
            g.wait_ge(ls, 160)
            g.wait_ge(pv, T)
            g.local_scatter(fin_sb[:, 0:1568], cdat_sb[:], c1x_sb[:],
                            channels=128, num_elems=1568,
                            num_idxs=CSTREAM).then_inc(gc, 1)
            g.local_scatter(fin_sb[:, 1568:3136], cdat_sb[:], c2x_sb[:],
                            channels=128, num_elems=1568,
                            num_idxs=CSTREAM).then_inc(gc, 1)
            g.wait_ge(vs, V_Q)
            g.dma_start(qown_v, qown_sb[:]).then_inc(ds, 16); d += 16
            g.wait_ge(ds, d)
            g.collective_compute(
                "AllGather", OP.bypass,
                replica_groups=[list(range(N_CORES))],
                ins=[qown[:, 0:1]],
                outs=[qtab[0:N_PAD, 0:1]],
            ).then_inc(ccs, 1)
            # ---- pass 2 ----
            g.wait_ge(ccs, 1)
            for k, (off, ln) in enumerate(FCHUNKS):
                bi = g.indirect_dma_start(
                    out=srow_sb[0:1, off:off + ln].rearrange(
                        "p (f one) -> p f one", one=1),
                    out_offset=None,
                    in_=qtab[:, :],
                    in_offset=IndirectOffsetOnAxis(
                        ap=fx_sb[:, off // 128:(off + ln) // 128],
                        axis=0),
                )
                bi.ins.queue = f"qPoolDynamic{(k % 4) or ''}"
                bi.then_inc(fs, 16); f += 16
            g.wait_ge(vs, V_D1)
            g.wait_ge(tp, T)
            g.local_scatter(aout_sb[:, 0:T1 * 128], data1_sb[:], a1x_sb[:],
                            channels=128, num_elems=T1 * 128,
                            num_idxs=2 * TCOLS).then_inc(ga, 1)
            g.local_scatter(aout_sb[:, T1 * 128:T * 128], data2_sb[:],
                            a2x_sb[:], channels=128, num_elems=T2 * 128,
                            num_idxs=4 * TCOLS).then_inc(ga, 1)
            g.wait_ge(fs, f)
            g.dma_start(fs2_row, srow_sb[:]).then_inc(fs, 16); f += 16
            g.wait_ge(pv, 2 * T)
            g.wait_ge(vs, V_SS1)
            g.local_scatter(fin_sb[:, 0:1568], cdat_sb[:], c1x_sb[:],
                            channels=128, num_elems=1568,
                            num_idxs=CSTREAM).then_inc(gc, 1)
            g.local_scatter(fin_sb[:, 1568:3136], cdat_sb[:], c2x_sb[:],
                            channels=128, num_elems=1568,
                            num_idxs=CSTREAM).then_inc(gc, 1)
            g.wait_ge(as_, 1)
            g.dma_start(outp_v, out_sb[:]).then_inc(ds, 16); d += 16
            g.wait_ge(ds, d)

        @block.tensor
        def _(t):
            t.wait_ge(ls, 80)
            for ph in range(2):
                for ti in range(T):
                    t.wait_ge(ga, 2 * ph + (1 if ti < T1 else 2))
                    k = ph * T + ti
                    if k >= 2:
                        t.wait_ge(pv, k - 1)
                    t.matmul(ps[k % 2][:],
                             aout_sb[:, ti * 128:(ti + 1) * 128],
                             id_sb[:], is_transpose=True).then_inc(tp, 1)

        @block.vector
        def _(v):
            # ---- pass 1 ----
            for ti in range(T):
                v.wait_ge(tp, ti + 1)
                v.tensor_scalar_add(cdat_sb[:, ti * 128:(ti + 1) * 128],
                                    ps[ti % 2][:], 0.0).then_inc(pv, 1)
            v.wait_ge(ls, 192)
            v.wait_ge(gc, 2)
            v.tensor_reduce(
                out=r1_sb[:],
                in_=fin_sb[:].rearrange("p (nv k) -> p nv k", k=K0),
                axis=mybir.AxisListType.X, op=OP.add)
            v.tensor_reduce(
                out=sraw_sb[:],
                in_=r1_sb[:, 0:2 * JPP].rearrange(
                    "p (j two) -> p j two", two=2),
                axis=mybir.AxisListType.X, op=OP.add)
            v.drain()
            v.tensor_tensor(out=sraw_sb[:, JPP - VF2:JPP],
                            in0=sraw_sb[:, JPP - VF2:JPP],
                            in1=r1_sb[:, 2 * JPP:2 * JPP + VF2],
                            op=OP.add)
            v.tensor_tensor(out=sraw_sb[:], in0=sraw_sb[:],
                            in1=pown_sb[:], op=OP.add)
            v.tensor_tensor(out=s1_sb[:], in0=sraw_sb[:],
                            in1=dinvo_sb[:], op=OP.mult).then_inc(vs, 1)
            for k in range(35):
                sig = sig_sbs[k % 4]
                v.wait_ge(pwa, k + 1)
                if k == 0:
                    v.tensor_scalar_mul(accA_sb[:], sig[:],
                                        w2_sb[:, 0:1]).then_inc(pwv, 1)
                else:
                    sa = accA_sb if k % 2 == 1 else accB_sb
                    da = accB_sb if k % 2 == 1 else accA_sb
                    v.scalar_tensor_tensor(
                        out=da[:], in0=sig[:], scalar=w2_sb[:, k:k + 1],
                        in1=sa[:], op0=OP.mult, op1=OP.add).then_inc(pwv, 1)
            v.tensor_tensor(out=qown_sb[:], in0=accA_sb[:],
                            in1=dinvo_sb[:], op=OP.mult).then_inc(vs, 1)
            # ---- pass 2 ----
            v.wait_ge(qs, 16)
            d1v = data1_sb[:].rearrange("p (x two) -> p x two", two=2)
            d2v = data2_sb[:].rearrange("p (x four) -> p x four", four=4)
            for j in range(2):
                v.tensor_scalar_add(d1v[:, :, j:j + 1], tab_sb[:], 0.0)
            for j in range(4):
                ins = v.tensor_scalar_add(d2v[:, :, j:j + 1], tab_sb[:], 0.0)
            ins.then_inc(vs, 1)                      # V_D1
            for ti in range(T):
                k = T + ti
                v.wait_ge(tp, k + 1)
                v.tensor_scalar_add(cdat_sb[:, ti * 128:(ti + 1) * 128],
                                    ps[k % 2][:], 0.0).then_inc(pv, 1)
            v.wait_ge(f2s, 32)
            v.tensor_scalar_add(cdat_sb[:, T * 128:T * 128 + WF1],
                                fstr1_sb[:], 0.0)
            v.tensor_scalar_add(fin_sb[:, 3136:3136 + WF2],
                                fstr2_sb[:], 0.0).then_inc(vs, 1)  # V_SS1
            v.wait_ge(gc, 4)
            v.tensor_reduce(
                out=r1_sb[:],
                in_=fin_sb[:].rearrange("p (nv k) -> p nv k", k=K0),
                axis=mybir.AxisListType.X, op=OP.add)
            v.tensor_reduce(
                out=sraw_sb[:],
                in_=r1_sb[:, 0:2 * JPP].rearrange(
                    "p (j two) -> p j two", two=2),
                axis=mybir.AxisListType.X, op=OP.add)
            v.drain()
            v.tensor_tensor(out=sraw_sb[:, JPP - VF2:JPP],
                            in0=sraw_sb[:, JPP - VF2:JPP],
                            in1=r1_sb[:, 2 * JPP:2 * JPP + VF2],
                            op=OP.add)
            v.tensor_tensor(out=sraw_sb[:], in0=sraw_sb[:],
                            in1=qown_sb[:], op=OP.add)
            v.tensor_tensor(out=s1_sb[:], in0=sraw_sb[:],
                            in1=dinvo_sb[:], op=OP.mult).then_inc(vs, 1)

        @block.scalar
        def _(a):
            a.wait_ge(ls, 256)
            a.wait_ge(vs, V_SIG1)
            for k in range(35):
                buf = sig_sbs[k % 4]
                if k >= 4:
                    a.wait_ge(pwv, k - 3)
                a.activation(buf[:], s1_sb[:], AF.Sigmoid,
                             bias=bb1_sb[:, k:k + 1],
                             scale=w1_sb[:, k:k + 1]).then_inc(pwa, 1)
            a.wait_ge(vs, V_S2)
            a.activation(out_sb[:], s1_sb[:], AF.Sigmoid,
                         bias=bb2_sb[:, 0:1]).then_inc(as_, 1)

    nc.compile()
    return nc


def kernel(x, edge_index, W1, b1, W2, b2):
    global LAST_RESULT
    from concourse.bass_utils import run_bass_kernel_spmd

    meta = _prep(x, edge_index)
    nc = _build_program(meta)
    in_maps = make_inmaps(meta, W1, b1, W2, b2)

    trace = os.environ.get("BASS_KERNEL_TRACE", "0") == "1"
    res = run_bass_kernel_spmd(nc, in_maps, list(range(N_CORES)), trace=trace)
    LAST_RESULT = res
    return unshard(meta, [res.results[c]["outp"] for c in range(N_CORES)])
